# revision 2
# baseline (speedup 1.0000x reference)
"""Distributed Trainium2 attention kernel (8 NeuronCores, head-sharded TP).

Reference computation (per problem spec, hardcoded shapes):
  B=2, S=2048, HID=2048, H=32 q-heads, HKV=8 kv-heads, D=64, GQA ratio 4.
  q/k/v projections -> RoPE(q,k) -> causal softmax attention -> out proj wo.

Sharding: tensor-parallel over heads; core i owns q-heads 4i..4i+3 and
kv-head i. Two AllToAlls convert head-sharding -> seq-sharding before the
row-split wo matmul. All on-device compute in transposed [feature, seq]
layout.

Design (driven by real NTFF traces; per-execution span ~450us vs ~610us
for the phase-sequential baseline):
  - The exec span = cross-core launch skew (absorbed by the first A2A's
    entry wait) + one core's serial work; everything here minimizes the
    serial work and keeps the tensor engine dense so its clock never
    re-throttles (HAM): sparse matmul streams run at ~1GHz effective,
    dense ones at 2.4GHz.
  - Software-pipelined attention stream with 2-tile scores lookahead
    across q-chunk boundaries; causal trimming of scores/exp/PV to the
    valid column range (no memsets needed).
  - Batch-1 projection is dissolved into the first attention scope as
    per-tile PE filler thunks; wo even-chunk tiles are woven through the
    hp1 blocks the same way. The ACT-bound exp stream paces attention
    while the PE stays busy on independent work.
  - Divide tail: PV stationary is [ones | 63-pad | v] so the softmax
    denominator lands on PSUM partition 0 (the only base the approx-recip
    custom DVE op supports; builtin reciprocal costs 3.3us per [1,512]
    single-lane slice, approx ~0.7us) and v-outputs land 64:128 (32-bank
    aligned for the DVE). Reciprocal is broadcast across partitions with
    a [1,128] fp16 ones-column matmul on the PE (a DRAM-bounce broadcast
    round-trip head-of-line blocks the DVE queue). Each block's divide is
    deferred into the next block so the PE never waits for it.
  - aot loads ride the sync HWDGE queue: on the ACT queue their
    collective-gated start head-of-line blocks the exp stream; the gpsimd
    SWDGE path is too slow/late. wo (8.4MB) loads on the scalar queue
    during attention.
  - 2 A2As (8-core, ~10-20us each): #1 fires after all hp0 blocks and
    overlaps hp1 attention; #2 gates only the wo odd-chunk tiles. Output
    is written bf16 (host casts back to f32).
"""

import os
import sys

import numpy as np
import ml_dtypes

sys.path.insert(0, "/opt/trn_rl_repo")

import concourse.bass as bass  # noqa: E402
import concourse.mybir as mybir  # noqa: E402
import concourse.tile as tile  # noqa: E402
from concourse import bacc  # noqa: E402
from concourse.bass_utils import run_bass_kernel_spmd  # noqa: E402

F32 = mybir.dt.float32
BF16 = mybir.dt.bfloat16

H, HKV, D = 32, 8, 64
HID = 2048
B, S = 2, 2048
BS = B * S            # 4096 flattened (b, s)
NCORES = 8
NHQ = H // NCORES     # 4 local q heads
MQ = NHQ * D          # 256 local q rows
SCHUNK = BS // NCORES  # 512 output rows per core

LAST_EXEC_NS = None


def _build(reps=1):
    nc = bacc.Bacc("TRN2", target_bir_lowering=False, debug=False,
                   num_devices=NCORES)

    hidden4 = nc.dram_tensor("hidden4", [8, 16, 128, 512], BF16,
                             kind="ExternalInput")
    wqT = nc.dram_tensor("wqT", [HID, MQ], BF16, kind="ExternalInput")
    wkvT = nc.dram_tensor("wkvT", [HID, 2 * D], BF16, kind="ExternalInput")
    woT = nc.dram_tensor("woT", [H * D, HID], BF16, kind="ExternalInput")
    cosT = nc.dram_tensor("cosT", [128, S], BF16, kind="ExternalInput")
    sinT = nc.dram_tensor("sinT", [128, S], BF16, kind="ExternalInput")
    maskb = nc.dram_tensor("maskb", [128, 2, 128], BF16, kind="ExternalInput")
    ident = nc.dram_tensor("ident", [128, D], BF16, kind="ExternalInput")
    out = nc.dram_tensor("out", [SCHUNK, HID], BF16, kind="ExternalOutput")

    from concourse.tile import add_dep_helper

    with tile.TileContext(nc) as tc:
        with (
            tc.tile_pool(name="persist", bufs=1) as persist,
            tc.tile_pool(name="qkv", bufs=1) as qkv,
            tc.tile_pool(name="dram", bufs=1, space="DRAM") as dram,
        ):
            # ---- persistent SBUF loads -------------------------------------
            wq_sb = persist.tile([128, 16, MQ], BF16)
            wqr = wqT.rearrange("(c p) m -> p c m", p=128)
            nc.sync.dma_start(wq_sb[:, 0:2, :], wqr[:, 0:2, :])
            nc.sync.dma_start(wq_sb[:, 2:16, :], wqr[:, 2:16, :])
            wkv_sb = persist.tile([128, 16, 2 * D], BF16)
            nc.scalar.dma_start(wkv_sb,
                                wkvT.rearrange("(c p) m -> p c m", p=128))
            cos_sb = persist.tile([128, S], BF16)
            cos_dma = nc.sync.dma_start(cos_sb, cosT[:, :])
            sin_sb = persist.tile([128, S], BF16)
            sin_dma = nc.sync.dma_start(sin_sb, sinT[:, :])
            mask_sb = persist.tile([128, 2, 128], BF16)
            nc.sync.dma_start(mask_sb, maskb[:, :, :])
            id_sb = persist.tile([128, D], BF16)
            nc.sync.dma_start(id_sb, ident[:, :])
            ones128 = persist.tile([1, 128], mybir.dt.float16)
            nc.vector.memset(ones128, 1.0)

            # ---- qkv outputs ----------------------------------------------
            qrope = qkv.tile([128, 2, BS], BF16)     # [64*(h%2), h//2, b*S+s]
            krope = qkv.tile([128, BS], BF16)        # duplicated both halves
            vT_sb = qkv.tile([128, BS], BF16)        # rows 64:128 used
            # per k-tile block: cols 0:64 = v, col 64 = ones (denominator)
            # per k-tile 96-wide stationary: col 0 = ones (denominator on
            # PSUM partition 0, where the approx-recip custom op works),
            # cols 1:32 zero pad (v outputs land 32-aligned for the DVE),
            # cols 32:96 = v
            v_sb = qkv.tile([128, B, 16 * 128], BF16)
            nc.vector.memset(v_sb, 0.0)
            for b in range(B):
                for kt in range(16):
                    nc.vector.memset(v_sb[:, b, kt * 128: kt * 128 + 1],
                                     1.0)

            cc_in_a = dram.tile([NCORES, 128, 512], BF16)  # heads 0,1
            cc_in_b = dram.tile([NCORES, 128, 512], BF16)  # heads 2,3
            cc_out_a = dram.tile([NCORES * 128, 512], BF16)
            cc_out_b = dram.tile([NCORES * 128, 512], BF16)

            for rep in range(reps):
                h_dmas = []

                # ======== projection segment: groups gs..gs+3 (batch b) =====
                def proj_segment(b):
                    with (
                        tc.tile_pool(name=f"hstream{b}", bufs=2) as hstream,
                        tc.tile_pool(name=f"ropetmp{b}", bufs=2) as ropetmp,
                        tc.tile_pool(name=f"psA{b}", bufs=1,
                                     space="PSUM") as psA,
                        tc.tile_pool(name=f"psT{b}", bufs=1,
                                     space="PSUM") as psT,
                    ):
                        for g in range(4 * b, 4 * b + 4):
                            cols = bass.ds(g * 512, 512)
                            ps_q0 = psA.tile([128, 512], F32, tag="q0")
                            ps_q1 = psA.tile([128, 512], F32, tag="q1")
                            ps_kv = psA.tile([128, 512], F32, tag="kv")
                            h_sb = hstream.tile([128, 16, 512], BF16, tag="h")
                            if g == 0:
                                nc.scalar.dma_start(
                                    h_sb[:, 0:4, :], hidden4[g, 0:4, :, :]
                                    .rearrange("c p n -> p c n"))
                                nc.scalar.dma_start(
                                    h_sb[:, 4:8, :], hidden4[g, 4:8, :, :]
                                    .rearrange("c p n -> p c n"))
                                h_dma = nc.scalar.dma_start(
                                    h_sb[:, 8:16, :], hidden4[g, 8:16, :, :]
                                    .rearrange("c p n -> p c n"))
                            else:
                                h_dma = nc.scalar.dma_start(
                                    h_sb, hidden4[g, :, :, :]
                                    .rearrange("c p n -> p c n"))
                            h_dmas.append(h_dma)
                            for c in range(16):
                                nc.tensor.matmul(ps_q0, wq_sb[:, c, 0:128],
                                                 h_sb[:, c, :],
                                                 start=(c == 0), stop=(c == 15))
                                nc.tensor.matmul(ps_q1, wq_sb[:, c, 128:256],
                                                 h_sb[:, c, :],
                                                 start=(c == 0), stop=(c == 15))
                                nc.tensor.matmul(ps_kv, wkv_sb[:, c, :],
                                                 h_sb[:, c, :],
                                                 start=(c == 0), stop=(c == 15))
                            # rope for q0/q1/kv batched
                            x = ropetmp.tile([128, 3, 512], BF16, tag="x")
                            nc.scalar.copy(x[:, 0, :], ps_q0)
                            nc.scalar.copy(x[:, 1, :], ps_q1)
                            nc.scalar.copy(x[0:64, 2, :], ps_kv[0:64, :])
                            swap = ropetmp.tile([128, 3, 512], BF16,
                                                tag="swap")
                            for blk in range(2):
                                p0 = blk * 64
                                nc.scalar.dma_start(swap[p0: p0 + 32, :, :],
                                                    x[p0 + 32: p0 + 64, :, :])
                                nc.scalar.dma_start(
                                    swap[p0 + 32: p0 + 64, :, :],
                                    x[p0: p0 + 32, :, :])
                            tmp = ropetmp.tile([128, 3, 512], BF16, tag="tmp")
                            cosp = ropetmp.tile([128, 3, 512], BF16,
                                                tag="cosp")
                            scols = bass.ds((g % 4) * 512, 512)
                            for t, rows in ((0, 128), (1, 128), (2, 64)):
                                nc.vector.tensor_mul(tmp[:rows, t, :],
                                                     swap[:rows, t, :],
                                                     sin_sb[:rows, scols])
                                nc.vector.tensor_mul(cosp[:rows, t, :],
                                                     x[:rows, t, :],
                                                     cos_sb[:rows, scols])
                            nc.vector.tensor_add(qrope[:, 0, cols],
                                                 tmp[:, 0, :], cosp[:, 0, :])
                            nc.vector.tensor_add(qrope[:, 1, cols],
                                                 tmp[:, 1, :], cosp[:, 1, :])
                            nc.vector.tensor_add(krope[0:64, cols],
                                                 tmp[0:64, 2, :],
                                                 cosp[0:64, 2, :])
                            nc.scalar.copy(vT_sb[64:128, cols],
                                           ps_kv[64:128, :])
                            nc.sync.dma_start(krope[64:128, cols],
                                              krope[0:64, cols])
                            for j in range(4):
                                st = g * 512 + j * 128
                                kt = (st - b * S) // 128
                                tp = psT.tile([128, D], BF16, tag="tp")
                                nc.tensor.transpose(
                                    tp, vT_sb[64:128, bass.ds(st, 128)],
                                    id_sb[64:128, :])
                                nc.vector.tensor_copy(
                                    v_sb[:, b, kt * 128 + 64: kt * 128 + 128],
                                    tp)
                        if rep == 0 and b == 0:
                            from concourse.tile import add_dep_helper as _adh
                            _adh(cos_dma.ins, h_dmas[0].ins, sync=True,
                                 reason="cos table after first hidden group")
                            _adh(sin_dma.ins, h_dmas[1].ins, sync=True,
                                 reason="sin table after second hidden group")

                # ======== attention block (hp, b, qc) =======================
                # emits scores/exp/mask/PV + divide + cc write for one q-chunk
                pending_div = []

                def flush_div():
                    # divide tail of the previous block: reciprocal of the
                    # denominator row, PE broadcast across partitions (f32r
                    # ones-column matmul), multiply, ship to the cc buffer.
                    if not pending_div:
                        return
                    psO, divp, osbs, cc_tile, slot = pending_div.pop()
                    recip = divp.tile([1, 1024], mybir.dt.float16,
                                      tag="recip", bufs=1, name="recip")
                    rec32 = divp.tile([1, 512], F32, tag="rec32",
                                      bufs=1, name="rec32")
                    # builtin DVE reciprocal costs ~3.3us on a [1,512]
                    # single-partition slice; the approx custom op is ~5x
                    # faster and exact to ~3e-6.  (It only works on
                    # partition-0-based APs, hence the ones column sits at
                    # index 0 so the denominator lands on PSUM partition 0.)
                    with nc.allow_low_precision(
                            reason="fp16 recip feeds the fp16 ones-matmul "
                                   "broadcast; 10-bit mantissa is ample"):
                        for half in range(2):
                            nc.vector.reciprocal_approx_fast(
                                rec32, osbs[half][0:1, :])
                            nc.vector.tensor_copy(
                                recip[0:1, bass.ts(half, 512)], rec32)
                    for half in range(2):
                        rbc = psO.tile([128, 512], F32, tag="rbc",
                                       name=f"rbc_{half}")
                        nc.tensor.matmul(
                            rbc,
                            ones128[0:1, :],
                            recip[0:1, bass.ts(half, 512)],
                            start=True, stop=True)
                        ao = divp.tile([128, 512], BF16, tag=f"ao{half}",
                                       name=f"ao_{half}", bufs=2)
                        nc.vector.tensor_mul(ao[64:128, :],
                                             osbs[half][64:128, :],
                                             rbc[64:128, :])
                        nc.sync.dma_start(
                            cc_tile[slot, bass.ts(half, 64), :],
                            ao[64:128, :])

                def attn_stream(tcp, blocks, filler=None,
                                after_block=None):
                    """Emit a software-pipelined attention stream.

                    blocks: list of (hp, b, qc). Scores are emitted with a
                    2-tile lookahead across block boundaries so the PE never
                    head-of-line blocks on the exp of a block's last tiles.
                    filler(bi) -> thunk or None: called after every tile; the
                    thunk emits a small slab of independent PE work (proj
                    matmuls, wo-even chunks) to keep the PE dense while the
                    ACT-bound exp stream paces attention. after_block: block
                    index -> callback (used to fire the first A2A).
                    """
                    psS, psO, psW, attnp, divp = tcp
                    seq = []
                    for bi, (hp, b, qc) in enumerate(blocks):
                        for kt in range(4 * qc + 4):
                            seq.append((bi, hp, b, qc, kt))
                    pss = {}

                    def emit_scores(i):
                        bi, hp, b, qc, kt = seq[i]
                        j = kt - 4 * qc
                        lo = max(j, 0) * 128
                        kcols = bass.ds(b * S + kt * 128, 128)
                        qc2 = bass.ds(b * S + qc * 512 + lo, 512 - lo)
                        ps_s = psS.tile([128, 2, 512], F32, tag="pss",
                                        name=f"ps_s_{hp}_{b}_{qc}_{kt}")
                        nc.tensor.matmul(ps_s[:, 0, lo:],
                                         krope[0:64, kcols],
                                         qrope[0:64, hp, qc2],
                                         start=True, stop=True)
                        nc.tensor.matmul(ps_s[:, 1, lo:],
                                         krope[64:128, kcols],
                                         qrope[64:128, hp, qc2],
                                         start=True, stop=True)
                        pss[i] = ps_s

                    LOOK = 2
                    for i in range(min(LOOK, len(seq))):
                        emit_scores(i)
                    pso = {}
                    for i, (bi, hp, b, qc, kt) in enumerate(seq):
                        if i + LOOK < len(seq):
                            emit_scores(i + LOOK)
                        nkt = 4 * qc + 4
                        if kt == 0:
                            # previous block's divide tail lands here, after
                            # this block's first scores are already in flight
                            flush_div()
                            pso[bi] = (psO.tile([128, 512], F32, tag="poa",
                                                name=f"poa_{bi}"),
                                       psO.tile([128, 512], F32, tag="pob",
                                                name=f"pob_{bi}"))
                        ps_oa, ps_ob = pso[bi]
                        ps_s = pss.pop(i)
                        j = kt - 4 * qc
                        lo = max(j, 0) * 128
                        attn = attnp.tile([128, 2, 512], BF16, tag="attn")
                        nc.scalar.activation(
                            attn[:, :, lo:], ps_s[:, :, lo:],
                            mybir.ActivationFunctionType.Exp,
                            scale=0.125)
                        if j >= 0:
                            nc.vector.tensor_mul(
                                attn[:, :, lo:lo + 128],
                                attn[:, :, lo:lo + 128], mask_sb)
                        vs = v_sb[:, b, kt * 128: kt * 128 + 128]
                        nc.tensor.matmul(ps_oa[:, lo:], vs, attn[:, 0, lo:],
                                         start=(kt == 0),
                                         stop=(kt == nkt - 1),
                                         skip_group_check=True)
                        nc.tensor.matmul(ps_ob[:, lo:], vs, attn[:, 1, lo:],
                                         start=(kt == 0),
                                         stop=(kt == nkt - 1),
                                         skip_group_check=True)
                        if filler is not None:
                            th = filler(bi)
                            if th is not None:
                                th()
                        if kt == nkt - 1:
                            cc_tile = cc_in_a if hp == 0 else cc_in_b
                            osbs = []
                            for half, ps_o in ((0, ps_oa), (1, ps_ob)):
                                osb = divp.tile([128, 512], F32,
                                                tag=f"osb{half}",
                                                name=f"osb_{half}")
                                nc.vector.tensor_copy(osb, ps_o)
                                osbs.append(osb)
                            pending_div.append((psO, divp, osbs, cc_tile,
                                                b * 4 + qc))
                            if after_block and bi in after_block:
                                after_block[bi]()

                # ================= emission ================================
                proj_segment(0)

                with (
                    tc.tile_pool(name="attnp", bufs=6) as attnp,
                    tc.tile_pool(name="divp", bufs=2) as divp,
                ):
                    # scope1: attention (hp0, b0) paced by ACT exp, with
                    # batch-1 projection woven in as per-tile PE filler --
                    # keeps the tensor engine dense (clock stays unthrottled)
                    # and dissolves the serial proj segment entirely.
                    with (
                        tc.tile_pool(name="psS1", bufs=2,
                                     space="PSUM") as psS,
                        tc.tile_pool(name="psO1", bufs=1,
                                     space="PSUM") as psO,  # poa/pob/rbc
                        tc.tile_pool(name="psF1", bufs=1,
                                     space="PSUM") as psF,
                        tc.tile_pool(name="hstream1", bufs=2) as hstream,
                        tc.tile_pool(name="ropetmp1", bufs=2) as ropetmp,
                    ):
                        import collections as _c
                        pstate = {}

                        def t_dma(g):
                            def t():
                                h_sb = hstream.tile([128, 16, 512], BF16,
                                                    tag="h",
                                                    name=f"h_sb_{g}")
                                pstate[("h", g)] = h_sb
                                h_dmas.append(nc.scalar.dma_start(
                                    h_sb, hidden4[g, :, :, :]
                                    .rearrange("c p n -> p c n")))
                            return t

                        def t_mm(g, tgt, cp):
                            def t():
                                if cp == 0:
                                    pstate[("pf", g, tgt)] = psF.tile(
                                        [128, 512], F32, tag="pf",
                                        name=f"pf_{g}_{tgt}")
                                ps = pstate[("pf", g, tgt)]
                                h_sb = pstate[("h", g)]
                                for c in (2 * cp, 2 * cp + 1):
                                    if tgt == 0:
                                        w = wq_sb[:, c, 0:128]
                                    elif tgt == 1:
                                        w = wq_sb[:, c, 128:256]
                                    else:
                                        w = wkv_sb[:, c, :]
                                    nc.tensor.matmul(
                                        ps, w, h_sb[:, c, :],
                                        start=(c == 0), stop=(c == 15),
                                        skip_group_check=True)
                            return t

                        def t_copy(g, tgt):
                            def t():
                                if tgt == 0:
                                    pstate[("x", g)] = ropetmp.tile(
                                        [128, 3, 512], BF16, tag="x",
                                        name=f"x_{g}")
                                x = pstate[("x", g)]
                                ps = pstate[("pf", g, tgt)]
                                if tgt < 2:
                                    nc.scalar.copy(x[:, tgt, :], ps)
                                else:
                                    nc.scalar.copy(x[0:64, 2, :],
                                                   ps[0:64, :])
                                    cols = bass.ds(g * 512, 512)
                                    nc.scalar.copy(vT_sb[64:128, cols],
                                                   ps[64:128, :])
                            return t

                        def t_rope(g, part):
                            def t():
                                cols = bass.ds(g * 512, 512)
                                x = pstate[("x", g)]
                                if part == 0:
                                    swap = ropetmp.tile([128, 3, 512], BF16,
                                                        tag="swap",
                                                        name=f"swap_{g}")
                                    pstate[("swap", g)] = swap
                                    for blk in range(2):
                                        p0 = blk * 64
                                        nc.scalar.dma_start(
                                            swap[p0: p0 + 32, :, :],
                                            x[p0 + 32: p0 + 64, :, :])
                                        nc.scalar.dma_start(
                                            swap[p0 + 32: p0 + 64, :, :],
                                            x[p0: p0 + 32, :, :])
                                elif part == 1:
                                    swap = pstate[("swap", g)]
                                    tmp = ropetmp.tile([128, 3, 512], BF16,
                                                       tag="tmp",
                                                       name=f"tmp_{g}")
                                    pstate[("tmp", g)] = tmp
                                    scols = bass.ds((g % 4) * 512, 512)
                                    for tt, rows in ((0, 128), (1, 128),
                                                     (2, 64)):
                                        nc.vector.tensor_mul(
                                            tmp[:rows, tt, :],
                                            swap[:rows, tt, :],
                                            sin_sb[:rows, scols])
                                else:
                                    tmp = pstate[("tmp", g)]
                                    cosp = ropetmp.tile([128, 3, 512], BF16,
                                                        tag="cosp",
                                                        name=f"cosp_{g}")
                                    scols = bass.ds((g % 4) * 512, 512)
                                    for tt, rows in ((0, 128), (1, 128),
                                                     (2, 64)):
                                        nc.vector.tensor_mul(
                                            cosp[:rows, tt, :],
                                            x[:rows, tt, :],
                                            cos_sb[:rows, scols])
                                    nc.vector.tensor_add(
                                        qrope[:, 0, cols], tmp[:, 0, :],
                                        cosp[:, 0, :])
                                    nc.vector.tensor_add(
                                        qrope[:, 1, cols], tmp[:, 1, :],
                                        cosp[:, 1, :])
                                    nc.vector.tensor_add(
                                        krope[0:64, cols], tmp[0:64, 2, :],
                                        cosp[0:64, 2, :])
                                    nc.sync.dma_start(krope[64:128, cols],
                                                      krope[0:64, cols])
                            return t

                        def t_tr(g, j):
                            def t():
                                st = g * 512 + j * 128
                                kt = (st - S) // 128
                                tp = psF.tile([128, D], BF16, tag="pf",
                                              name=f"tp_{g}_{j}")
                                nc.tensor.transpose(
                                    tp, vT_sb[64:128, bass.ds(st, 128)],
                                    id_sb[64:128, :])
                                nc.vector.tensor_copy(
                                    v_sb[:, 1, kt * 128 + 64: kt * 128 + 128],
                                    tp)
                            return t

                        thunks = _c.deque()
                        thunks.append(t_dma(4))
                        thunks.append(t_dma(5))
                        for g in range(4, 8):
                            for tgt in range(3):
                                for cp in range(8):
                                    thunks.append(t_mm(g, tgt, cp))
                                thunks.append(t_copy(g, tgt))
                            for part in range(3):
                                thunks.append(t_rope(g, part))
                            for j in range(4):
                                thunks.append(t_tr(g, j))
                            if g + 2 < 8:
                                thunks.append(t_dma(g + 2))

                        tcp = (psS, psO, None, attnp, divp)
                        attn_stream(tcp, [(0, 0, qc) for qc in range(4)],
                                    filler=lambda bi: (thunks.popleft()
                                                       if thunks else None))
                        while thunks:
                            thunks.popleft()()
                        flush_div()
                    wop = ctx_wop = tc.tile_pool(name="wop", bufs=1)
                    wop = ctx_wop.__enter__()
                    wo_sb = wop.tile([128, 16, HID], BF16)
                    for ch in range(4):
                        nc.scalar.dma_start(
                            wo_sb[:, bass.ts(ch, 4), :],
                            woT.rearrange("(c p) n -> p c n",
                                          p=128)[:, bass.ts(ch, 4), :])
                    ev_sb = wop.tile([128, 4, HID], BF16)
                    with (
                        tc.tile_pool(name="psS2", bufs=2,
                                     space="PSUM") as psS,
                        tc.tile_pool(name="psO2", bufs=1,
                                     space="PSUM") as psO,  # poa/pob/rbc
                        tc.tile_pool(name="psW", bufs=1,
                                     space="PSUM") as psW,
                    ):
                        tcp = (psS, psO, psW, attnp, divp)

                        def fire_a2a1():
                            flush_div()
                            nc.gpsimd.collective_compute(
                                "AllToAll", mybir.AluOpType.bypass,
                                replica_groups=[list(range(NCORES))],
                                ins=[cc_in_a.opt()],
                                outs=[cc_out_a.opt()])
                            nc.sync.dma_start(
                                aot_a,
                                cc_out_a.rearrange("(c p) n -> p c n",
                                                   p=128))

                        aot_a = wop.tile([128, 8, 512], BF16)
                        # wo-even chunk thunks: woven one per tile through
                        # the hp1 blocks (PE density filler; aot_a is ready
                        # by then for the late cores, and early cores have
                        # slack while they wait out the A2A skew anyway)
                        import collections as _c2
                        estate = {}

                        def t_ev(st, nh, ii):
                            def t():
                                ns = bass.ts(nh, 512)
                                if ii == 0:
                                    estate[(st, nh)] = psW.tile(
                                        [128, 512], F32, tag="psw",
                                        name=f"ps_e_{st}_{nh}")
                                ps_e = estate[(st, nh)]
                                for k in (2 * ii, 2 * ii + 1):
                                    nc.tensor.matmul(
                                        ps_e, aot_a[:, k, bass.ts(st, 128)],
                                        wo_sb[:, 2 * k, ns],
                                        start=(k == 0), stop=(k == 7),
                                        skip_group_check=True)
                            return t

                        def t_evc(st, nh):
                            def t():
                                ns = bass.ts(nh, 512)
                                nc.vector.tensor_copy(ev_sb[:, st, ns],
                                                      estate[(st, nh)])
                            return t

                        evq = _c2.deque()
                        for st in range(4):
                            for nh in range(4):
                                for ii in range(4):
                                    evq.append(t_ev(st, nh, ii))
                                evq.append(t_evc(st, nh))

                        blocks = ([(0, 1, qc) for qc in range(4)]
                                  + [(1, b, qc) for b in range(B)
                                     for qc in range(4)])
                        attn_stream(tcp, blocks,
                                    filler=lambda bi: (evq.popleft()
                                                       if bi >= 5 and evq
                                                       else None),
                                    after_block={3: fire_a2a1})
                        flush_div()
                        nc.gpsimd.collective_compute(
                            "AllToAll", mybir.AluOpType.bypass,
                            replica_groups=[list(range(NCORES))],
                            ins=[cc_in_b.opt()],
                            outs=[cc_out_b.opt()])
                        aot_b = wop.tile([128, 8, 512], BF16)
                        ccob = cc_out_b.rearrange("(c p) n -> p c n", p=128)
                        nc.sync.dma_start(aot_b[:, 0:2, :], ccob[:, 0:2, :])
                        nc.sync.dma_start(aot_b[:, 2:8, :], ccob[:, 2:8, :])
                        while evq:
                            evq.popleft()()

                    # ============= wo odd chunks + merge ====================
                    with (
                        tc.tile_pool(name="psWo", bufs=2, space="PSUM") as psWo,
                        tc.tile_pool(name="outp", bufs=2) as outp,
                    ):
                        for st in range(4):
                            ps_w = psWo.tile([128, HID], F32, tag="psw",
                                             name=f"ps_o_{st}")
                            for i in range(8):
                                for nh in range(4):
                                    ns = bass.ts(nh, 512)
                                    nc.tensor.matmul(
                                        ps_w[:, ns],
                                        aot_b[:, i, bass.ts(st, 128)],
                                        wo_sb[:, 2 * i + 1, ns],
                                        start=(i == 0), stop=(i == 7))
                            osb = outp.tile([128, HID], BF16, tag="osb")
                            nc.vector.tensor_add(osb, ps_w, ev_sb[:, st, :])
                            nc.sync.dma_start(out[bass.ts(st, 128), :], osb)
                    ctx_wop.__exit__(None, None, None)

    nc.compile()
    return nc


_NC_CACHE = {}


def _get_nc(reps=1):
    key = f"nc{reps}"
    if key not in _NC_CACHE:
        _NC_CACHE[key] = _build(reps)
    return _NC_CACHE[key]


def _prep_inputs(hidden_states, cos, sin, wq, wk, wv, wo):
    bf = ml_dtypes.bfloat16
    hiddenT = np.ascontiguousarray(
        hidden_states.reshape(BS, HID).T).astype(bf)       # [HID, BS]
    hidden4 = np.ascontiguousarray(
        hiddenT.reshape(16, 128, 8, 512).transpose(2, 0, 1, 3))
    woT = np.ascontiguousarray(np.asarray(wo).T).astype(bf)

    cos2 = np.asarray(cos)[:, 0, :]          # [S, D]
    sin2 = np.asarray(sin)[:, 0, :]
    cosTb = cos2.T                            # [D, S]
    sinTb = sin2.T
    sin_signed = np.concatenate([-sinTb[:32], sinTb[32:]], axis=0)
    cos_full = np.tile(cosTb, (2, 1)).astype(bf)       # [128, S]
    sin_full = np.tile(sin_signed, (2, 1)).astype(bf)  # [128, S]

    # triangular causal band mask, duplicated for the two heads of a pair
    kk = np.arange(128)[:, None]
    qq = np.arange(128)[None, :]
    maskb1 = np.where(kk > qq, 0.0, 1.0).astype(np.float32).astype(bf)
    maskb = np.ascontiguousarray(
        np.broadcast_to(maskb1[:, None, :], (128, 2, 128)))

    ident_np = np.zeros((128, D), np.float32)
    ident_np[64:128, :] = np.eye(D)
    ident_np = ident_np.astype(bf)

    wq = np.asarray(wq)
    wk = np.asarray(wk)
    wv = np.asarray(wv)
    in_maps = []
    for i in range(NCORES):
        wq_i = wq[i * MQ:(i + 1) * MQ, :]                      # [256, HID]
        wkv_i = np.concatenate([wk[i * D:(i + 1) * D, :],
                                wv[i * D:(i + 1) * D, :]], axis=0)
        in_maps.append({
            "hidden4": hidden4,
            "wqT": np.ascontiguousarray(wq_i.T).astype(bf),
            "wkvT": np.ascontiguousarray(wkv_i.T).astype(bf),
            "woT": woT,
            "cosT": cos_full,
            "sinT": sin_full,
            "maskb": maskb,
            "ident": ident_np,
        })
    return in_maps


def kernel(hidden_states, cos, sin, wq, wk, wv, wo):
    global LAST_EXEC_NS
    reps = int(os.environ.get("KREPS", "1"))
    in_maps = _prep_inputs(np.asarray(hidden_states, np.float32),
                           cos, sin, wq, wk, wv, wo)
    nc = _get_nc(reps)
    res = run_bass_kernel_spmd(nc, in_maps, core_ids=list(range(NCORES)),
                               trace=bool(int(os.environ.get("BASS_TRACE",
                                                             "0"))))
    LAST_EXEC_NS = res.exec_time_ns
    outs = [res.results[i]["out"].astype(np.float32) for i in range(NCORES)]
    full = np.concatenate(outs, axis=0).reshape(B, S, HID)
    return full


# revision 3
# speedup vs baseline: 1.1203x; 1.1203x over previous
"""Distributed Trainium2 attention kernel (8 NeuronCores, head-sharded TP).

Reference computation (per problem spec, hardcoded shapes):
  B=2, S=2048, HID=2048, H=32 q-heads, HKV=8 kv-heads, D=64, GQA ratio 4.
  q/k/v projections -> RoPE(q,k) -> causal softmax attention -> out proj wo.

Sharding: tensor-parallel over heads; core i owns q-heads 4i..4i+3 and
kv-head i. Two AllToAlls convert head-sharding -> seq-sharding before the
row-split wo matmul. All on-device compute in transposed [feature, seq]
layout.

Design (driven by real NTFF traces; per-execution span ~450us vs ~610us
for the phase-sequential baseline):
  - The exec span = cross-core launch skew (absorbed by the first A2A's
    entry wait) + one core's serial work; everything here minimizes the
    serial work and keeps the tensor engine dense so its clock never
    re-throttles (HAM): sparse matmul streams run at ~1GHz effective,
    dense ones at 2.4GHz.
  - Software-pipelined attention stream with 2-tile scores lookahead
    across q-chunk boundaries; causal trimming of scores/exp/PV to the
    valid column range (no memsets needed).
  - Batch-1 projection is dissolved into the first attention scope as
    per-tile PE filler thunks; wo even-chunk tiles are woven through the
    hp1 blocks the same way. The ACT-bound exp stream paces attention
    while the PE stays busy on independent work.
  - Divide tail: PV stationary is [ones | 63-pad | v] so the softmax
    denominator lands on PSUM partition 0 (the only base the approx-recip
    custom DVE op supports; builtin reciprocal costs 3.3us per [1,512]
    single-lane slice, approx ~0.7us) and v-outputs land 64:128 (32-bank
    aligned for the DVE). Reciprocal is broadcast across partitions with
    a [1,128] fp16 ones-column matmul on the PE (a DRAM-bounce broadcast
    round-trip head-of-line blocks the DVE queue). Each block's divide is
    deferred into the next block so the PE never waits for it.
  - aot loads ride the sync HWDGE queue: on the ACT queue their
    collective-gated start head-of-line blocks the exp stream; the gpsimd
    SWDGE path is too slow/late. wo (8.4MB) loads on the scalar queue
    during attention.
  - 2 A2As (8-core, ~10-20us each): #1 fires after all hp0 blocks and
    overlaps hp1 attention; #2 gates only the wo odd-chunk tiles. Output
    is written bf16 (host casts back to f32).
"""

import os
import sys

import numpy as np
import ml_dtypes

sys.path.insert(0, "/opt/trn_rl_repo")

import concourse.bass as bass  # noqa: E402
import concourse.mybir as mybir  # noqa: E402
import concourse.tile as tile  # noqa: E402
from concourse import bacc  # noqa: E402
from concourse.bass_utils import run_bass_kernel_spmd  # noqa: E402

F32 = mybir.dt.float32
BF16 = mybir.dt.bfloat16

H, HKV, D = 32, 8, 64
HID = 2048
B, S = 2, 2048
BS = B * S            # 4096 flattened (b, s)
NCORES = 8
NHQ = H // NCORES     # 4 local q heads
MQ = NHQ * D          # 256 local q rows
SCHUNK = BS // NCORES  # 512 output rows per core

LAST_EXEC_NS = None


def _build(reps=1):
    nc = bacc.Bacc("TRN2", target_bir_lowering=False, debug=False,
                   num_devices=NCORES)

    hidden4 = nc.dram_tensor("hidden4", [8, 16, 128, 512], BF16,
                             kind="ExternalInput")
    wqT = nc.dram_tensor("wqT", [HID, MQ], BF16, kind="ExternalInput")
    wkvT = nc.dram_tensor("wkvT", [HID, 2 * D], BF16, kind="ExternalInput")
    woT = nc.dram_tensor("woT", [H * D, HID], BF16, kind="ExternalInput")
    cosT = nc.dram_tensor("cosT", [128, S], BF16, kind="ExternalInput")
    sinT = nc.dram_tensor("sinT", [128, S], BF16, kind="ExternalInput")
    maskb = nc.dram_tensor("maskb", [128, 2, 128], BF16, kind="ExternalInput")
    ident = nc.dram_tensor("ident", [128, D], BF16, kind="ExternalInput")
    out = nc.dram_tensor("out", [SCHUNK, HID], BF16, kind="ExternalOutput")

    from concourse.tile import add_dep_helper

    with tile.TileContext(nc) as tc:
        with (
            tc.tile_pool(name="persist", bufs=1) as persist,
            tc.tile_pool(name="qkv", bufs=1) as qkv,
            tc.tile_pool(name="dram", bufs=1, space="DRAM") as dram,
        ):
            # ---- persistent SBUF loads -------------------------------------
            wq_sb = persist.tile([128, 16, MQ], BF16)
            wqr = wqT.rearrange("(c p) m -> p c m", p=128)
            nc.sync.dma_start(wq_sb[:, 0:2, :], wqr[:, 0:2, :])
            nc.sync.dma_start(wq_sb[:, 2:16, :], wqr[:, 2:16, :])
            wkv_sb = persist.tile([128, 16, 2 * D], BF16)
            nc.scalar.dma_start(wkv_sb,
                                wkvT.rearrange("(c p) m -> p c m", p=128))
            cos_sb = persist.tile([128, S], BF16)
            cos_dma = nc.sync.dma_start(cos_sb, cosT[:, :])
            sin_sb = persist.tile([128, S], BF16)
            sin_dma = nc.sync.dma_start(sin_sb, sinT[:, :])
            mask_sb = persist.tile([128, 2, 128], BF16)
            nc.sync.dma_start(mask_sb, maskb[:, :, :])
            id_sb = persist.tile([128, D], BF16)
            nc.sync.dma_start(id_sb, ident[:, :])
            ones128 = persist.tile([1, 128], mybir.dt.float16)
            nc.vector.memset(ones128, 1.0)

            # ---- qkv outputs ----------------------------------------------
            qrope = qkv.tile([128, 2, BS], BF16)     # [64*(h%2), h//2, b*S+s]
            krope = qkv.tile([128, BS], BF16)        # duplicated both halves
            vT_sb = qkv.tile([128, BS], BF16)        # rows 64:128 used
            # per k-tile block: cols 0:64 = v, col 64 = ones (denominator)
            # per k-tile 96-wide stationary: col 0 = ones (denominator on
            # PSUM partition 0, where the approx-recip custom op works),
            # cols 1:32 zero pad (v outputs land 32-aligned for the DVE),
            # cols 32:96 = v
            v_sb = qkv.tile([128, B, 16 * 128], BF16)
            nc.vector.memset(v_sb, 0.0)
            for b in range(B):
                for kt in range(16):
                    nc.vector.memset(v_sb[:, b, kt * 128: kt * 128 + 1],
                                     1.0)

            cc_in_a = dram.tile([NCORES, 128, 512], BF16)  # heads 0,1
            cc_in_b = dram.tile([NCORES, 128, 512], BF16)  # heads 2,3
            cc_out_a = dram.tile([NCORES * 128, 512], BF16)
            cc_out_b = dram.tile([NCORES * 128, 512], BF16)

            for rep in range(reps):
                h_dmas = []

                # ======== projection segment: groups gs..gs+3 (batch b) =====
                def proj_segment(b):
                    with (
                        tc.tile_pool(name=f"hstream{b}", bufs=2) as hstream,
                        tc.tile_pool(name=f"ropetmp{b}", bufs=2) as ropetmp,
                        tc.tile_pool(name=f"psA{b}", bufs=1,
                                     space="PSUM") as psA,
                        tc.tile_pool(name=f"psT{b}", bufs=1,
                                     space="PSUM") as psT,
                    ):
                        for g in range(4 * b, 4 * b + 4):
                            cols = bass.ds(g * 512, 512)
                            ps_q0 = psA.tile([128, 512], F32, tag="q0")
                            ps_q1 = psA.tile([128, 512], F32, tag="q1")
                            ps_kv = psA.tile([128, 512], F32, tag="kv")
                            h_sb = hstream.tile([128, 16, 512], BF16, tag="h")
                            if g == 0:
                                nc.scalar.dma_start(
                                    h_sb[:, 0:4, :], hidden4[g, 0:4, :, :]
                                    .rearrange("c p n -> p c n"))
                                nc.scalar.dma_start(
                                    h_sb[:, 4:8, :], hidden4[g, 4:8, :, :]
                                    .rearrange("c p n -> p c n"))
                                h_dma = nc.scalar.dma_start(
                                    h_sb[:, 8:16, :], hidden4[g, 8:16, :, :]
                                    .rearrange("c p n -> p c n"))
                            else:
                                h_dma = nc.scalar.dma_start(
                                    h_sb, hidden4[g, :, :, :]
                                    .rearrange("c p n -> p c n"))
                            h_dmas.append(h_dma)
                            for c in range(16):
                                nc.tensor.matmul(ps_q0, wq_sb[:, c, 0:128],
                                                 h_sb[:, c, :],
                                                 start=(c == 0), stop=(c == 15))
                                nc.tensor.matmul(ps_q1, wq_sb[:, c, 128:256],
                                                 h_sb[:, c, :],
                                                 start=(c == 0), stop=(c == 15))
                                nc.tensor.matmul(ps_kv, wkv_sb[:, c, :],
                                                 h_sb[:, c, :],
                                                 start=(c == 0), stop=(c == 15))
                            # rope for q0/q1/kv batched
                            x = ropetmp.tile([128, 3, 512], BF16, tag="x")
                            nc.scalar.copy(x[:, 0, :], ps_q0)
                            nc.scalar.copy(x[:, 1, :], ps_q1)
                            nc.scalar.copy(x[0:64, 2, :], ps_kv[0:64, :])
                            swap = ropetmp.tile([128, 3, 512], BF16,
                                                tag="swap")
                            for blk in range(2):
                                p0 = blk * 64
                                nc.scalar.dma_start(swap[p0: p0 + 32, :, :],
                                                    x[p0 + 32: p0 + 64, :, :])
                                nc.scalar.dma_start(
                                    swap[p0 + 32: p0 + 64, :, :],
                                    x[p0: p0 + 32, :, :])
                            tmp = ropetmp.tile([128, 3, 512], BF16, tag="tmp")
                            cosp = ropetmp.tile([128, 3, 512], BF16,
                                                tag="cosp")
                            scols = bass.ds((g % 4) * 512, 512)
                            for t, rows in ((0, 128), (1, 128), (2, 64)):
                                nc.vector.tensor_mul(tmp[:rows, t, :],
                                                     swap[:rows, t, :],
                                                     sin_sb[:rows, scols])
                                nc.vector.tensor_mul(cosp[:rows, t, :],
                                                     x[:rows, t, :],
                                                     cos_sb[:rows, scols])
                            nc.vector.tensor_add(qrope[:, 0, cols],
                                                 tmp[:, 0, :], cosp[:, 0, :])
                            nc.vector.tensor_add(qrope[:, 1, cols],
                                                 tmp[:, 1, :], cosp[:, 1, :])
                            nc.vector.tensor_add(krope[0:64, cols],
                                                 tmp[0:64, 2, :],
                                                 cosp[0:64, 2, :])
                            nc.scalar.copy(vT_sb[64:128, cols],
                                           ps_kv[64:128, :])
                            nc.sync.dma_start(krope[64:128, cols],
                                              krope[0:64, cols])
                            for j in range(4):
                                st = g * 512 + j * 128
                                kt = (st - b * S) // 128
                                tp = psT.tile([128, D], BF16, tag="tp")
                                nc.tensor.transpose(
                                    tp, vT_sb[64:128, bass.ds(st, 128)],
                                    id_sb[64:128, :])
                                nc.vector.tensor_copy(
                                    v_sb[:, b, kt * 128 + 64: kt * 128 + 128],
                                    tp)
                        if rep == 0 and b == 0:
                            from concourse.tile import add_dep_helper as _adh
                            _adh(cos_dma.ins, h_dmas[0].ins, sync=True,
                                 reason="cos table after first hidden group")
                            _adh(sin_dma.ins, h_dmas[1].ins, sync=True,
                                 reason="sin table after second hidden group")

                # ======== attention block (hp, b, qc) =======================
                # emits scores/exp/mask/PV + divide + cc write for one q-chunk
                pending_div = []

                def flush_div():
                    # divide tail of the previous block: reciprocal of the
                    # denominator row, PE broadcast across partitions (f32r
                    # ones-column matmul), multiply, ship to the cc buffer.
                    if not pending_div:
                        return
                    psO, divp, osbs, cc_tile, slot = pending_div.pop()
                    recip = divp.tile([1, 1024], mybir.dt.float16,
                                      tag="recip", bufs=1, name="recip")
                    rec32 = divp.tile([1, 512], F32, tag="rec32",
                                      bufs=1, name="rec32")
                    # builtin DVE reciprocal costs ~3.3us on a [1,512]
                    # single-partition slice; the approx custom op is ~5x
                    # faster and exact to ~3e-6.  (It only works on
                    # partition-0-based APs, hence the ones column sits at
                    # index 0 so the denominator lands on PSUM partition 0.)
                    with nc.allow_low_precision(
                            reason="fp16 recip feeds the fp16 ones-matmul "
                                   "broadcast; 10-bit mantissa is ample"):
                        for half in range(2):
                            nc.vector.reciprocal_approx_fast(
                                rec32, osbs[half][0:1, :])
                            nc.vector.tensor_copy(
                                recip[0:1, bass.ts(half, 512)], rec32)
                    for half in range(2):
                        rbc = psO.tile([128, 512], F32, tag="rbc",
                                       name=f"rbc_{half}")
                        nc.tensor.matmul(
                            rbc,
                            ones128[0:1, :],
                            recip[0:1, bass.ts(half, 512)],
                            start=True, stop=True)
                        ao = divp.tile([128, 512], BF16, tag=f"ao{half}",
                                       name=f"ao_{half}", bufs=2)
                        nc.vector.tensor_mul(ao[64:128, :],
                                             osbs[half][64:128, :],
                                             rbc[64:128, :])
                        nc.sync.dma_start(
                            cc_tile[slot, bass.ts(half, 64), :],
                            ao[64:128, :])

                def attn_stream(tcp, blocks, filler=None,
                                after_block=None):
                    """Emit a software-pipelined attention stream.

                    blocks: list of (hp, b, qc). Scores are emitted with a
                    2-tile lookahead across block boundaries so the PE never
                    head-of-line blocks on the exp of a block's last tiles.
                    filler(bi) -> thunk or None: called after every tile; the
                    thunk emits a small slab of independent PE work (proj
                    matmuls, wo-even chunks) to keep the PE dense while the
                    ACT-bound exp stream paces attention. after_block: block
                    index -> callback (used to fire the first A2A).
                    """
                    psS, psO, psW, attnp, divp = tcp
                    seq = []
                    for bi, (hp, b, qc) in enumerate(blocks):
                        for kt in range(4 * qc + 4):
                            seq.append((bi, hp, b, qc, kt))
                    pss = {}

                    def emit_scores(i):
                        bi, hp, b, qc, kt = seq[i]
                        j = kt - 4 * qc
                        lo = max(j, 0) * 128
                        kcols = bass.ds(b * S + kt * 128, 128)
                        qc2 = bass.ds(b * S + qc * 512 + lo, 512 - lo)
                        ps_s = psS.tile([128, 2, 512], F32, tag="pss",
                                        name=f"ps_s_{hp}_{b}_{qc}_{kt}")
                        nc.tensor.matmul(ps_s[:, 0, lo:],
                                         krope[0:64, kcols],
                                         qrope[0:64, hp, qc2],
                                         start=True, stop=True)
                        nc.tensor.matmul(ps_s[:, 1, lo:],
                                         krope[64:128, kcols],
                                         qrope[64:128, hp, qc2],
                                         start=True, stop=True)
                        pss[i] = ps_s

                    LOOK = 2
                    for i in range(min(LOOK, len(seq))):
                        emit_scores(i)
                    pso = {}
                    for i, (bi, hp, b, qc, kt) in enumerate(seq):
                        if i + LOOK < len(seq):
                            emit_scores(i + LOOK)
                        nkt = 4 * qc + 4
                        if kt == 0:
                            # previous block's divide tail lands here, after
                            # this block's first scores are already in flight
                            flush_div()
                            pso[bi] = (psO.tile([128, 512], F32, tag="poa",
                                                name=f"poa_{bi}"),
                                       psO.tile([128, 512], F32, tag="pob",
                                                name=f"pob_{bi}"))
                        ps_oa, ps_ob = pso[bi]
                        ps_s = pss.pop(i)
                        j = kt - 4 * qc
                        lo = max(j, 0) * 128
                        attn = attnp.tile([128, 2, 512], BF16, tag="attn")
                        nc.scalar.activation(
                            attn[:, :, lo:], ps_s[:, :, lo:],
                            mybir.ActivationFunctionType.Exp,
                            scale=0.125)
                        if j >= 0:
                            nc.vector.tensor_mul(
                                attn[:, :, lo:lo + 128],
                                attn[:, :, lo:lo + 128], mask_sb)
                        vs = v_sb[:, b, kt * 128: kt * 128 + 128]
                        nc.tensor.matmul(ps_oa[:, lo:], vs, attn[:, 0, lo:],
                                         start=(kt == 0),
                                         stop=(kt == nkt - 1),
                                         skip_group_check=True)
                        nc.tensor.matmul(ps_ob[:, lo:], vs, attn[:, 1, lo:],
                                         start=(kt == 0),
                                         stop=(kt == nkt - 1),
                                         skip_group_check=True)
                        if filler is not None:
                            th = filler(bi)
                            if th is not None:
                                th()
                        if kt == nkt - 1:
                            cc_tile = cc_in_a if hp == 0 else cc_in_b
                            osbs = []
                            for half, ps_o in ((0, ps_oa), (1, ps_ob)):
                                osb = divp.tile([128, 512], F32,
                                                tag=f"osb{half}",
                                                name=f"osb_{half}")
                                nc.vector.tensor_copy(osb, ps_o)
                                osbs.append(osb)
                            pending_div.append((psO, divp, osbs, cc_tile,
                                                b * 4 + qc))
                            if after_block and bi in after_block:
                                after_block[bi]()

                # ================= emission ================================
                proj_segment(0)

                with (
                    tc.tile_pool(name="attnp", bufs=6) as attnp,
                    tc.tile_pool(name="divp", bufs=2) as divp,
                ):
                    # scope1: attention (hp0, b0) paced by ACT exp, with
                    # batch-1 projection woven in as per-tile PE filler --
                    # keeps the tensor engine dense (clock stays unthrottled)
                    # and dissolves the serial proj segment entirely.
                    with (
                        tc.tile_pool(name="psS1", bufs=2,
                                     space="PSUM") as psS,
                        tc.tile_pool(name="psO1", bufs=1,
                                     space="PSUM") as psO,  # poa/pob/rbc
                        tc.tile_pool(name="psF1", bufs=1,
                                     space="PSUM") as psF,
                        tc.tile_pool(name="hstream1", bufs=2) as hstream,
                        tc.tile_pool(name="ropetmp1", bufs=2) as ropetmp,
                    ):
                        import collections as _c
                        pstate = {}

                        def t_dma(g):
                            def t():
                                h_sb = hstream.tile([128, 16, 512], BF16,
                                                    tag="h",
                                                    name=f"h_sb_{g}")
                                pstate[("h", g)] = h_sb
                                h_dmas.append(nc.scalar.dma_start(
                                    h_sb, hidden4[g, :, :, :]
                                    .rearrange("c p n -> p c n")))
                            return t

                        def t_mm(g, tgt, cp):
                            def t():
                                if cp == 0:
                                    pstate[("pf", g, tgt)] = psF.tile(
                                        [128, 512], F32, tag="pf",
                                        name=f"pf_{g}_{tgt}")
                                ps = pstate[("pf", g, tgt)]
                                h_sb = pstate[("h", g)]
                                for c in (2 * cp, 2 * cp + 1):
                                    if tgt == 0:
                                        w = wq_sb[:, c, 0:128]
                                    elif tgt == 1:
                                        w = wq_sb[:, c, 128:256]
                                    else:
                                        w = wkv_sb[:, c, :]
                                    nc.tensor.matmul(
                                        ps, w, h_sb[:, c, :],
                                        start=(c == 0), stop=(c == 15),
                                        skip_group_check=True)
                            return t

                        def t_copy(g, tgt):
                            def t():
                                if tgt == 0:
                                    pstate[("x", g)] = ropetmp.tile(
                                        [128, 3, 512], BF16, tag="x",
                                        name=f"x_{g}")
                                x = pstate[("x", g)]
                                ps = pstate[("pf", g, tgt)]
                                if tgt < 2:
                                    nc.scalar.copy(x[:, tgt, :], ps)
                                else:
                                    nc.scalar.copy(x[0:64, 2, :],
                                                   ps[0:64, :])
                                    cols = bass.ds(g * 512, 512)
                                    nc.scalar.copy(vT_sb[64:128, cols],
                                                   ps[64:128, :])
                            return t

                        def t_rope(g, part):
                            def t():
                                cols = bass.ds(g * 512, 512)
                                x = pstate[("x", g)]
                                if part == 0:
                                    swap = ropetmp.tile([128, 3, 512], BF16,
                                                        tag="swap",
                                                        name=f"swap_{g}")
                                    pstate[("swap", g)] = swap
                                    for blk in range(2):
                                        p0 = blk * 64
                                        nc.scalar.dma_start(
                                            swap[p0: p0 + 32, :, :],
                                            x[p0 + 32: p0 + 64, :, :])
                                        nc.scalar.dma_start(
                                            swap[p0 + 32: p0 + 64, :, :],
                                            x[p0: p0 + 32, :, :])
                                elif part == 1:
                                    swap = pstate[("swap", g)]
                                    tmp = ropetmp.tile([128, 3, 512], BF16,
                                                       tag="tmp",
                                                       name=f"tmp_{g}")
                                    pstate[("tmp", g)] = tmp
                                    scols = bass.ds((g % 4) * 512, 512)
                                    for tt, rows in ((0, 128), (1, 128),
                                                     (2, 64)):
                                        nc.vector.tensor_mul(
                                            tmp[:rows, tt, :],
                                            swap[:rows, tt, :],
                                            sin_sb[:rows, scols])
                                else:
                                    tmp = pstate[("tmp", g)]
                                    cosp = ropetmp.tile([128, 3, 512], BF16,
                                                        tag="cosp",
                                                        name=f"cosp_{g}")
                                    scols = bass.ds((g % 4) * 512, 512)
                                    for tt, rows in ((0, 128), (1, 128),
                                                     (2, 64)):
                                        nc.vector.tensor_mul(
                                            cosp[:rows, tt, :],
                                            x[:rows, tt, :],
                                            cos_sb[:rows, scols])
                                    nc.vector.tensor_add(
                                        qrope[:, 0, cols], tmp[:, 0, :],
                                        cosp[:, 0, :])
                                    nc.vector.tensor_add(
                                        qrope[:, 1, cols], tmp[:, 1, :],
                                        cosp[:, 1, :])
                                    nc.vector.tensor_add(
                                        krope[0:64, cols], tmp[0:64, 2, :],
                                        cosp[0:64, 2, :])
                                    nc.sync.dma_start(krope[64:128, cols],
                                                      krope[0:64, cols])
                            return t

                        def t_tr(g, j):
                            def t():
                                st = g * 512 + j * 128
                                kt = (st - S) // 128
                                tp = psF.tile([128, D], BF16, tag="pf",
                                              name=f"tp_{g}_{j}")
                                nc.tensor.transpose(
                                    tp, vT_sb[64:128, bass.ds(st, 128)],
                                    id_sb[64:128, :])
                                nc.vector.tensor_copy(
                                    v_sb[:, 1, kt * 128 + 64: kt * 128 + 128],
                                    tp)
                            return t

                        thunks = _c.deque()
                        thunks.append(t_dma(4))
                        thunks.append(t_dma(5))
                        for g in range(4, 8):
                            for tgt in range(3):
                                for cp in range(8):
                                    thunks.append(t_mm(g, tgt, cp))
                                thunks.append(t_copy(g, tgt))
                            for part in range(3):
                                thunks.append(t_rope(g, part))
                            for j in range(4):
                                thunks.append(t_tr(g, j))
                            if g + 2 < 8:
                                thunks.append(t_dma(g + 2))

                        tcp = (psS, psO, None, attnp, divp)
                        attn_stream(tcp, [(0, 0, qc) for qc in range(4)],
                                    filler=lambda bi: (thunks.popleft()
                                                       if thunks else None))
                        while thunks:
                            thunks.popleft()()
                        flush_div()
                    wop = ctx_wop = tc.tile_pool(name="wop", bufs=1)
                    wop = ctx_wop.__enter__()
                    wo_sb = wop.tile([128, 16, HID], BF16)
                    for ch in range(4):
                        nc.scalar.dma_start(
                            wo_sb[:, bass.ts(ch, 4), :],
                            woT.rearrange("(c p) n -> p c n",
                                          p=128)[:, bass.ts(ch, 4), :])
                    ev_sb = wop.tile([128, 4, HID], BF16)
                    with (
                        tc.tile_pool(name="psS2", bufs=2,
                                     space="PSUM") as psS,
                        tc.tile_pool(name="psO2", bufs=1,
                                     space="PSUM") as psO,  # poa/pob/rbc
                        tc.tile_pool(name="psW", bufs=1,
                                     space="PSUM") as psW,
                    ):
                        tcp = (psS, psO, psW, attnp, divp)

                        def fire_a2a1():
                            flush_div()
                            nc.gpsimd.collective_compute(
                                "AllToAll", mybir.AluOpType.bypass,
                                replica_groups=[list(range(NCORES))],
                                ins=[cc_in_a.opt()],
                                outs=[cc_out_a.opt()])
                            nc.sync.dma_start(
                                aot_a,
                                cc_out_a.rearrange("(c p) n -> p c n",
                                                   p=128))

                        aot_a = wop.tile([128, 8, 512], BF16)
                        # wo-even chunk thunks: woven one per tile through
                        # the hp1 blocks (PE density filler; aot_a is ready
                        # by then for the late cores, and early cores have
                        # slack while they wait out the A2A skew anyway)
                        import collections as _c2
                        estate = {}

                        def t_ev(st, nh, ii):
                            def t():
                                ns = bass.ts(nh, 512)
                                if ii == 0:
                                    estate[(st, nh)] = psW.tile(
                                        [128, 512], F32, tag="psw",
                                        name=f"ps_e_{st}_{nh}")
                                ps_e = estate[(st, nh)]
                                for k in (2 * ii, 2 * ii + 1):
                                    nc.tensor.matmul(
                                        ps_e, aot_a[:, k, bass.ts(st, 128)],
                                        wo_sb[:, 2 * k, ns],
                                        start=(k == 0), stop=(k == 7),
                                        skip_group_check=True)
                            return t

                        def t_evc(st, nh):
                            def t():
                                ns = bass.ts(nh, 512)
                                nc.vector.tensor_copy(ev_sb[:, st, ns],
                                                      estate[(st, nh)])
                            return t

                        evq = _c2.deque()
                        for st in range(4):
                            for nh in range(4):
                                for ii in range(4):
                                    evq.append(t_ev(st, nh, ii))
                                evq.append(t_evc(st, nh))

                        blocks = ([(0, 1, qc) for qc in range(4)]
                                  + [(1, b, qc) for b in range(B)
                                     for qc in range(4)])
                        # consume ev thunks on alternate tiles only: the
                        # reserved half drains after the A2A#2 trigger and
                        # fills its ~15-20us flight window on every core
                        evctr = [0]

                        def ev_filler(bi):
                            if bi < 6 or not evq:
                                return None
                            evctr[0] += 1
                            if evctr[0] % 2:
                                return evq.popleft()
                            return None

                        attn_stream(tcp, blocks, filler=ev_filler,
                                    after_block={3: fire_a2a1})
                        flush_div()
                        nc.gpsimd.collective_compute(
                            "AllToAll", mybir.AluOpType.bypass,
                            replica_groups=[list(range(NCORES))],
                            ins=[cc_in_b.opt()],
                            outs=[cc_out_b.opt()])
                        aot_b = wop.tile([128, 8, 512], BF16)
                        ccob = cc_out_b.rearrange("(c p) n -> p c n", p=128)
                        nc.sync.dma_start(aot_b[:, 0:2, :], ccob[:, 0:2, :])
                        nc.sync.dma_start(aot_b[:, 2:8, :], ccob[:, 2:8, :])
                        while evq:
                            evq.popleft()()

                    # ============= wo odd chunks + merge ====================
                    with (
                        tc.tile_pool(name="psWo", bufs=2, space="PSUM") as psWo,
                        tc.tile_pool(name="outp", bufs=2) as outp,
                    ):
                        for st in range(4):
                            ps_w = psWo.tile([128, HID], F32, tag="psw",
                                             name=f"ps_o_{st}")
                            for i in range(8):
                                for nh in range(4):
                                    ns = bass.ts(nh, 512)
                                    nc.tensor.matmul(
                                        ps_w[:, ns],
                                        aot_b[:, i, bass.ts(st, 128)],
                                        wo_sb[:, 2 * i + 1, ns],
                                        start=(i == 0), stop=(i == 7))
                            osb = outp.tile([128, HID], BF16, tag="osb")
                            nc.vector.tensor_add(osb, ps_w, ev_sb[:, st, :])
                            nc.sync.dma_start(out[bass.ts(st, 128), :], osb)
                    ctx_wop.__exit__(None, None, None)

    nc.compile()
    return nc


_NC_CACHE = {}


def _get_nc(reps=1):
    key = f"nc{reps}"
    if key not in _NC_CACHE:
        _NC_CACHE[key] = _build(reps)
    return _NC_CACHE[key]


def _prep_inputs(hidden_states, cos, sin, wq, wk, wv, wo):
    bf = ml_dtypes.bfloat16
    hiddenT = np.ascontiguousarray(
        hidden_states.reshape(BS, HID).T).astype(bf)       # [HID, BS]
    hidden4 = np.ascontiguousarray(
        hiddenT.reshape(16, 128, 8, 512).transpose(2, 0, 1, 3))
    woT = np.ascontiguousarray(np.asarray(wo).T).astype(bf)

    cos2 = np.asarray(cos)[:, 0, :]          # [S, D]
    sin2 = np.asarray(sin)[:, 0, :]
    cosTb = cos2.T                            # [D, S]
    sinTb = sin2.T
    sin_signed = np.concatenate([-sinTb[:32], sinTb[32:]], axis=0)
    cos_full = np.tile(cosTb, (2, 1)).astype(bf)       # [128, S]
    sin_full = np.tile(sin_signed, (2, 1)).astype(bf)  # [128, S]

    # triangular causal band mask, duplicated for the two heads of a pair
    kk = np.arange(128)[:, None]
    qq = np.arange(128)[None, :]
    maskb1 = np.where(kk > qq, 0.0, 1.0).astype(np.float32).astype(bf)
    maskb = np.ascontiguousarray(
        np.broadcast_to(maskb1[:, None, :], (128, 2, 128)))

    ident_np = np.zeros((128, D), np.float32)
    ident_np[64:128, :] = np.eye(D)
    ident_np = ident_np.astype(bf)

    wq = np.asarray(wq)
    wk = np.asarray(wk)
    wv = np.asarray(wv)
    in_maps = []
    for i in range(NCORES):
        wq_i = wq[i * MQ:(i + 1) * MQ, :]                      # [256, HID]
        wkv_i = np.concatenate([wk[i * D:(i + 1) * D, :],
                                wv[i * D:(i + 1) * D, :]], axis=0)
        in_maps.append({
            "hidden4": hidden4,
            "wqT": np.ascontiguousarray(wq_i.T).astype(bf),
            "wkvT": np.ascontiguousarray(wkv_i.T).astype(bf),
            "woT": woT,
            "cosT": cos_full,
            "sinT": sin_full,
            "maskb": maskb,
            "ident": ident_np,
        })
    return in_maps


def kernel(hidden_states, cos, sin, wq, wk, wv, wo):
    global LAST_EXEC_NS
    reps = int(os.environ.get("KREPS", "1"))
    in_maps = _prep_inputs(np.asarray(hidden_states, np.float32),
                           cos, sin, wq, wk, wv, wo)
    nc = _get_nc(reps)
    res = run_bass_kernel_spmd(nc, in_maps, core_ids=list(range(NCORES)),
                               trace=bool(int(os.environ.get("BASS_TRACE",
                                                             "0"))))
    LAST_EXEC_NS = res.exec_time_ns
    outs = [res.results[i]["out"].astype(np.float32) for i in range(NCORES)]
    full = np.concatenate(outs, axis=0).reshape(B, S, HID)
    return full
